# revision 8
# baseline (speedup 1.0000x reference)
"""Adaptive LM head (3-tier chunked softmax cross-entropy) on 8 TRN2 NeuronCores.

Strategy: data-parallel over B_T = 8192 rows (1024 rows/core; weights
replicated). Per core:
  - logits for each tier computed as bf16 matmuls (hT stationary, W streaming,
    f32->bf16 cast done in-flight by SWDGE DMA), PSUM accumulation over the
    contraction dim, 2048-wide vocab super-tiles.
  - ScalarE activation(Exp, accum_out=...) fuses exp + per-row sum in a single
    pass over each [128, 2048] logit tile (accum_out -> per-row partial Z).
  - tier units are interleaved (PE-heavy tier0 against ACT-heavy tier2) so
    TensorE stays dense/warm while ScalarE drains exp sums.
  - target logit = dot(feature_row, W[:, target]) computed exactly in f32:
    indirect-DMA gather of transposed-weight rows + fused scalar_tensor_tensor
    multiply-reduce, spread across the main loop.
  - per-core partial loss (sum_rows(log Z - target_logit)/8192) is the output;
    the host sums the 8 partials (the unshard step for a DP loss).
"""

import numpy as np

from concourse import bacc, bass, mybir
from concourse.bass import IndirectOffsetOnAxis
from concourse.bass_utils import run_bass_kernel_spmd
from concourse.tile import TileContext

F32 = mybir.dt.float32
BF16 = mybir.dt.bfloat16
I32 = mybir.dt.int32
ALU = mybir.AluOpType
ACTF = mybir.ActivationFunctionType

P = 128
D = 1024
N_CORES = 8
RPC = 1024          # rows per core
NRT = RPC // P      # row tiles per core = 8
ST = 2048           # vocab super-tile width
NB = 512            # matmul free-dim tile (one PSUM bank)
NBM = 512           # main-loop moving-operand tile
V0, V1, V2 = 8192, 16384, 25681
PD1, PD2 = 256, 128
B_T = 8192

# Window schedule: each window pairs one PE-heavy unit with four ACT-heavy
# units, interleaved at row-tile granularity so the depth-2 PSUM ping-pong
# never blocks TensorE behind ScalarE's exp+sum drains.
WINDOWS = [
    ((0, 0), [(2, 0), (2, 1), (1, 0), (2, 2)]),
    ((0, 1), [(2, 3), (2, 4), (1, 1), (2, 5)]),
    ((0, 2), [(2, 6), (2, 7), (1, 2), (2, 8)]),
    ((0, 3), [(2, 9), (2, 10), (1, 3), (2, 11)]),
    ((1, 4), [(2, 12), (1, 5), (1, 6), (1, 7)]),
]

_NC_CACHE = None


def _ceil_div(a, b):
    return (a + b - 1) // b


def _build_graph():
    nc = bacc.Bacc("TRN2", target_bir_lowering=False, debug=False,
                   num_devices=N_CORES)

    ht_ext = nc.declare_dram_parameter("ht", [D, RPC], F32, isOutput=False)
    hr_ext = nc.declare_dram_parameter("hr", [RPC, D], F32, isOutput=False)
    tf_ext = nc.declare_dram_parameter("tf", [P, NRT], F32, isOutput=False)
    wp1_ext = nc.declare_dram_parameter("wp1", [D, PD1], F32, isOutput=False)
    wp2_ext = nc.declare_dram_parameter("wp2", [D, PD2], F32, isOutput=False)
    w0_ext = nc.declare_dram_parameter("w0", [D, V0], F32, isOutput=False)
    w1_ext = nc.declare_dram_parameter("w1", [PD1, V1], F32, isOutput=False)
    w2_ext = nc.declare_dram_parameter("w2", [PD2, V2], F32, isOutput=False)
    wt0_ext = nc.declare_dram_parameter("wt0", [V0, D], F32, isOutput=False)
    wt1_ext = nc.declare_dram_parameter("wt1", [V1, PD1], F32, isOutput=False)
    wt2_ext = nc.declare_dram_parameter("wt2", [V2, PD2], F32, isOutput=False)
    out_ext = nc.declare_dram_parameter("out", [1, 1], F32, isOutput=True)

    with TileContext(nc) as tc:
        with (
            tc.tile_pool(name="res", bufs=1) as res,
            tc.tile_pool(name="w0pool", bufs=2) as w0pool,
            tc.tile_pool(name="w1pool", bufs=3) as w1pool,
            tc.tile_pool(name="w2pool", bufs=4) as w2pool,
            tc.tile_pool(name="hrpool", bufs=2) as hrpool,
            tc.tile_pool(name="expool", bufs=3) as expool,
            tc.tile_pool(name="gpool", bufs=2) as gpool,
            tc.tile_pool(name="prodpool", bufs=2) as prodpool,
            tc.tile_pool(name="psum", bufs=2, space="PSUM") as psum,
        ):
            # ---------------- resident tiles ----------------
            ht_sb = res.tile([P, 8 * RPC], BF16, tag="ht")       # 8 d-chunks
            wp1_sb = res.tile([P, 8 * PD1], BF16, tag="wp1")
            wp2_sb = res.tile([P, 8 * PD2], BF16, tag="wp2")
            hp1T_sb = res.tile([P, 2 * RPC], BF16, tag="hp1T")
            hp2T_sb = res.tile([P, 1 * RPC], BF16, tag="hp2T")
            hp1r_sb = res.tile([P, NRT * PD1], F32, tag="hp1r")
            hp2r_sb = res.tile([P, NRT * PD2], F32, tag="hp2r")
            tf_sb = res.tile([P, NRT], F32, tag="tf")
            ge1 = res.tile([P, NRT], F32, tag="ge1")
            ge2 = res.tile([P, NRT], F32, tag="ge2")
            idxf = [res.tile([P, NRT], F32, tag=f"idxf{t}", name=f"idxf{t}")
                    for t in range(3)]
            idxi = [res.tile([P, NRT], I32, tag=f"idxi{t}", name=f"idxi{t}")
                    for t in range(3)]
            tl = [res.tile([P, NRT], F32, tag=f"tl{t}", name=f"tl{t}")
                  for t in range(3)]
            zbig = res.tile([P, NRT * 32], F32, tag="zbig")
            zred = res.tile([P, NRT], F32, tag="zred")
            logz = res.tile([P, NRT], F32, tag="logz")
            d1 = res.tile([P, NRT], F32, tag="d1")
            d2 = res.tile([P, NRT], F32, tag="d2")
            loss8 = res.tile([P, NRT], F32, tag="loss8")
            lossv = res.tile([P, 1], F32, tag="lossv")
            ones = res.tile([P, 1], F32, tag="ones")
            part = res.tile([1, 1], F32, tag="part")

            # ---------------- input staging ----------------
            # order matters for scheduling priority: ht (feeds everything),
            # then the first units' W slices arrive via the per-tier pools
            for k in range(8):
                nc.gpsimd.dma_start(
                    out=ht_sb[:, k * RPC:(k + 1) * RPC],
                    in_=ht_ext[k * P:(k + 1) * P, :])
            nc.sync.dma_start(out=tf_sb[:], in_=tf_ext[:, :])
            for k in range(8):
                nc.gpsimd.dma_start(
                    out=wp1_sb[:, k * PD1:(k + 1) * PD1],
                    in_=wp1_ext[k * P:(k + 1) * P, :])
                nc.gpsimd.dma_start(
                    out=wp2_sb[:, k * PD2:(k + 1) * PD2],
                    in_=wp2_ext[k * P:(k + 1) * P, :])

            nc.vector.memset(zbig[:], 0.0)
            nc.vector.memset(ones[:], 1.0)

            # ---------------- masks and in-tier indices ----------------
            nc.vector.tensor_scalar(out=ge1[:], in0=tf_sb[:], scalar1=float(V0),
                                    scalar2=None, op0=ALU.is_ge)
            nc.vector.tensor_scalar(out=ge2[:], in0=tf_sb[:],
                                    scalar1=float(V0 + V1), scalar2=None,
                                    op0=ALU.is_ge)
            nc.vector.tensor_scalar(out=idxf[0][:], in0=tf_sb[:],
                                    scalar1=float(V0 - 1), scalar2=None,
                                    op0=ALU.min)
            nc.vector.tensor_scalar(out=idxf[1][:], in0=tf_sb[:],
                                    scalar1=-float(V0), scalar2=0.0,
                                    op0=ALU.add, op1=ALU.max)
            nc.vector.tensor_scalar(out=idxf[1][:], in0=idxf[1][:],
                                    scalar1=float(V1 - 1), scalar2=None,
                                    op0=ALU.min)
            nc.vector.tensor_scalar(out=idxf[2][:], in0=tf_sb[:],
                                    scalar1=-float(V0 + V1), scalar2=0.0,
                                    op0=ALU.add, op1=ALU.max)
            nc.vector.tensor_scalar(out=idxf[2][:], in0=idxf[2][:],
                                    scalar1=float(V2 - 1), scalar2=None,
                                    op0=ALU.min)
            for t in range(3):
                nc.vector.tensor_copy(out=idxi[t][:], in_=idxf[t][:])

            # ---------------- projections ----------------
            for m in range(PD1 // P):
                for rb in range(RPC // NB):
                    ps = psum.tile([P, ST], F32, tag="ps")
                    for k in range(8):
                        nc.tensor.matmul(
                            out=ps[:, :NB],
                            lhsT=wp1_sb[:, k * PD1 + m * P: k * PD1 + (m + 1) * P],
                            rhs=ht_sb[:, k * RPC + rb * NB: k * RPC + (rb + 1) * NB],
                            start=(k == 0), stop=(k == 7))
                    nc.vector.tensor_copy(
                        out=hp1T_sb[:, m * RPC + rb * NB: m * RPC + (rb + 1) * NB],
                        in_=ps[:, :NB])
            for rb in range(RPC // NB):
                ps = psum.tile([P, ST], F32, tag="ps")
                for k in range(8):
                    nc.tensor.matmul(
                        out=ps[:, :NB],
                        lhsT=wp2_sb[:, k * PD2:(k + 1) * PD2],
                        rhs=ht_sb[:, k * RPC + rb * NB: k * RPC + (rb + 1) * NB],
                        start=(k == 0), stop=(k == 7))
                nc.vector.tensor_copy(
                    out=hp2T_sb[:, rb * NB:(rb + 1) * NB], in_=ps[:, :NB])

            for rt in range(NRT):
                ps = psum.tile([P, ST], F32, tag="ps")
                for k in range(8):
                    nc.tensor.matmul(
                        out=ps[:, :PD1],
                        lhsT=ht_sb[:, k * RPC + rt * P: k * RPC + rt * P + P],
                        rhs=wp1_sb[:, k * PD1:(k + 1) * PD1],
                        start=(k == 0), stop=(k == 7))
                nc.vector.tensor_copy(
                    out=hp1r_sb[:, rt * PD1:(rt + 1) * PD1], in_=ps[:, :PD1])
            for rt in range(NRT):
                ps = psum.tile([P, ST], F32, tag="ps")
                for k in range(8):
                    nc.tensor.matmul(
                        out=ps[:, :PD2],
                        lhsT=ht_sb[:, k * RPC + rt * P: k * RPC + rt * P + P],
                        rhs=wp2_sb[:, k * PD2:(k + 1) * PD2],
                        start=(k == 0), stop=(k == 7))
                nc.vector.tensor_copy(
                    out=hp2r_sb[:, rt * PD2:(rt + 1) * PD2], in_=ps[:, :PD2])

            # ---------------- interleaved main units ----------------
            tiers = {
                0: (V0, 8, w0_ext, ht_sb, w0pool, 8),
                1: (V1, 2, w1_ext, hp1T_sb, w1pool, 2),
                2: (V2, 1, w2_ext, hp2T_sb, w2pool, 1),
            }
            gather_src = [wt0_ext, wt1_ext, wt2_ext]
            gdim = [D, PD1, PD2]
            gmax = [V0 - 1, V1 - 1, V2 - 1]

            def emit_gather_dot(i):
                rt, t = divmod(i, 3)
                if t == 0:
                    hr_t = hrpool.tile([P, D], F32, tag="hrt", name="hrt")
                    nc.sync.dma_start(out=hr_t[:],
                                      in_=hr_ext[rt * P:(rt + 1) * P, :])
                    feat_ap = hr_t[:]
                elif t == 1:
                    feat_ap = hp1r_sb[:, rt * PD1:(rt + 1) * PD1]
                else:
                    feat_ap = hp2r_sb[:, rt * PD2:(rt + 1) * PD2]
                g = gpool.tile([P, gdim[t]], F32, tag=f"g{t}", name=f"g{t}")
                nc.gpsimd.indirect_dma_start(
                    out=g[:], out_offset=None,
                    in_=gather_src[t][:, :],
                    in_offset=IndirectOffsetOnAxis(
                        ap=idxi[t][:, rt:rt + 1], axis=0),
                    bounds_check=gmax[t], oob_is_err=False)
                prod = prodpool.tile([P, D], F32, tag="prod")
                nc.vector.scalar_tensor_tensor(
                    out=prod[:, :gdim[t]],
                    in0=feat_ap, scalar=1.0, in1=g[:],
                    op0=ALU.mult, op1=ALU.mult,
                    accum_out=tl[t][:, rt:rt + 1])

            unit_idx = {}
            unit_wtile = {}

            def ensure_unit(u):
                if u in unit_idx:
                    return
                tier, st = u
                V, K, w_ext, lhsT_sb, wpool, nchunks = tiers[tier]
                w = min(ST, V - st * ST)
                wtile = wpool.tile([P, nchunks * ST], BF16,
                                   tag=f"w{tier}", name=f"w{tier}")
                for k in range(K):
                    nc.gpsimd.dma_start(
                        out=wtile[:, k * ST: k * ST + w],
                        in_=w_ext[k * P:(k + 1) * P, st * ST: st * ST + w])
                unit_idx[u] = len(unit_idx)
                unit_wtile[u] = wtile

            def emit_rt(u, rt):
                tier, st = u
                V, K, w_ext, lhsT_sb, wpool, nchunks = tiers[tier]
                w = min(ST, V - st * ST)
                wtile = unit_wtile[u]
                ui = unit_idx[u]
                nb = _ceil_div(w, NBM)
                ps = psum.tile([P, ST], F32, tag="ps")
                for k in range(K):
                    for b in range(nb):
                        bw = min(NBM, w - b * NBM)
                        nc.tensor.matmul(
                            out=ps[:, b * NBM: b * NBM + bw],
                            lhsT=lhsT_sb[:, k * RPC + rt * P:
                                         k * RPC + rt * P + P],
                            rhs=wtile[:, k * ST + b * NBM:
                                      k * ST + b * NBM + bw],
                            start=(k == 0), stop=(k == K - 1))
                ex = expool.tile([P, ST], BF16, tag="ex")
                nc.scalar.activation(
                    ex[:, :w], ps[:, :w], ACTF.Exp,
                    accum_out=zbig[:, rt * 32 + ui: rt * 32 + ui + 1])

            slot_no = 0
            for heavy, lights in WINDOWS:
                ensure_unit(heavy)
                light_rts = [(lu, rt) for lu in lights for rt in range(NRT)]
                for lu in lights:
                    pass
                for i in range(NRT):
                    for lu, rt in light_rts[4 * i: 4 * i + 4]:
                        ensure_unit(lu)
                        emit_rt(lu, rt)
                    emit_rt(heavy, i)
                    if slot_no < 3 * NRT:
                        emit_gather_dot(slot_no)
                    slot_no += 1

            # ---------------- final reduction ----------------
            for rt in range(NRT):
                nc.vector.tensor_reduce(
                    out=zred[:, rt:rt + 1], in_=zbig[:, rt * 32:(rt + 1) * 32],
                    axis=mybir.AxisListType.X, op=ALU.add)
            nc.scalar.activation(logz[:], zred[:], ACTF.Ln)
            # loss8 = logz - (tl0 + ge1*(tl1-tl0) + ge2*(tl2-tl1))
            nc.vector.tensor_tensor(out=d1[:], in0=tl[1][:], in1=tl[0][:],
                                    op=ALU.subtract)
            nc.vector.tensor_tensor(out=d2[:], in0=tl[2][:], in1=tl[1][:],
                                    op=ALU.subtract)
            nc.vector.tensor_tensor(out=d1[:], in0=d1[:], in1=ge1[:],
                                    op=ALU.mult)
            nc.vector.tensor_tensor(out=d2[:], in0=d2[:], in1=ge2[:],
                                    op=ALU.mult)
            nc.vector.tensor_tensor(out=loss8[:], in0=logz[:], in1=tl[0][:],
                                    op=ALU.subtract)
            nc.vector.tensor_tensor(out=loss8[:], in0=loss8[:], in1=d1[:],
                                    op=ALU.subtract)
            nc.vector.tensor_tensor(out=loss8[:], in0=loss8[:], in1=d2[:],
                                    op=ALU.subtract)
            nc.vector.tensor_reduce(out=lossv[:], in_=loss8[:],
                                    axis=mybir.AxisListType.X, op=ALU.add)
            ps = psum.tile([P, ST], F32, tag="ps")
            nc.tensor.matmul(out=ps[0:1, 0:1], lhsT=lossv[:], rhs=ones[:],
                             start=True, stop=True)
            nc.scalar.mul(part[0:1, 0:1], ps[0:1, 0:1], 1.0 / float(B_T))
            nc.sync.dma_start(out=out_ext[:, :], in_=part[:])

    nc.compile()
    return nc


def _get_nc():
    global _NC_CACHE
    if _NC_CACHE is None:
        _NC_CACHE = _build_graph()
    return _NC_CACHE


def _make_in_maps(h, targets, W_head0, W_proj1, W_head1, W_proj2, W_head2):
    h = np.ascontiguousarray(np.asarray(h, dtype=np.float32)).reshape(B_T, D)
    t = np.asarray(targets).reshape(-1).astype(np.float32)
    w0 = np.ascontiguousarray(np.asarray(W_head0, dtype=np.float32))
    w1 = np.ascontiguousarray(np.asarray(W_head1, dtype=np.float32))
    w2 = np.ascontiguousarray(np.asarray(W_head2, dtype=np.float32))
    wp1 = np.ascontiguousarray(np.asarray(W_proj1, dtype=np.float32))
    wp2 = np.ascontiguousarray(np.asarray(W_proj2, dtype=np.float32))
    wt0 = np.ascontiguousarray(w0.T)
    wt1 = np.ascontiguousarray(w1.T)
    wt2 = np.ascontiguousarray(w2.T)

    in_maps = []
    for c in range(N_CORES):
        hc = h[c * RPC:(c + 1) * RPC]
        tc_ = t[c * RPC:(c + 1) * RPC]
        in_maps.append({
            "ht": np.ascontiguousarray(hc.T),
            "hr": hc,
            "tf": np.ascontiguousarray(tc_.reshape(NRT, P).T),
            "wp1": wp1, "wp2": wp2,
            "w0": w0, "w1": w1, "w2": w2,
            "wt0": wt0, "wt1": wt1, "wt2": wt2,
        })
    return in_maps


def kernel(h, targets, token_to_tier, token_to_idx,
           W_head0, W_proj1, W_head1, W_proj2, W_head2):
    in_maps = _make_in_maps(h, targets, W_head0, W_proj1, W_head1,
                            W_proj2, W_head2)
    nc = _get_nc()
    res = run_bass_kernel_spmd(nc, in_maps, core_ids=list(range(N_CORES)))
    total = sum(float(res.results[c]["out"][0, 0]) for c in range(N_CORES))
    return np.float32(total)
